# revision 24
# baseline (speedup 1.0000x reference)
"""Contour -> distance map kernel for 8 Trainium2 NeuronCores.

Math (per polygon, pixel m=(mx,my), edge k: vertex a=v_k, next b=v_{k+1}):
  cross_k = (a-m) x (b-m)   (affine in [1, mx, my])
  n2_k    = |a-m|^2         (affine in phi(m) = [1, mx, my, mx^2+my^2])
The reference's tanh/arccos winding-angle sum equals (a.e.) the integer
winding number, which the kernel computes by signed ray-crossing counting:
  W = 1/2 * sum_k tanh(1e5*cross_k) * h_k,
  h_k = [sgn(vy_k - my) != sgn(vy_{k+1} - my)]  (edge spans the pixel row).
h depends only on (edge, my); my has a fixed 128-lane pattern per pixel-tile
parity, so h is a host-precomputed constant SBUF table - no dot products,
reciprocals or arctans on device (CPU-checked rel err 2.7e-4 vs reference).

Sharding: core i handles polygon pair {2*(i//2), 2*(i//2)+1} on pixel half
i%2 (32768 px).  Per 128-pixel tile ONE fp32r matmul (1 cycle/col at >=256
out cols) yields psum[128, (poly, kind, 64)] = n2|cross for both polygons.
Then: ACT tanh(1e5*cross)->bf16, DVE c = t1*h (bf16 2x mode), DVE add-reduce
c -> winding sum, DVE min-reduce n2.  Host epilogue: wind = |SC|/2,
prod = wind*sqrt(MN), dmap = prod / global_max(prod).
"""

import numpy as np

SIZE = 256
NCORES = 8
K = 64
NPOLY = 2            # polygons per core
NT = 256             # 128-pixel tiles per core (half the image)
BATCH = 8            # tiles per psum batch
NBATCH = NT // BATCH

import os as _os
MM4 = _os.environ.get("ANT_MM4", "1") == "1"
MM_ORDER = [int(x) for x in _os.environ.get(
    "ANT_MM_ORDER", "0,1,2,3,4,5,6,7").split(",")]

_CACHE = {}


def _build_program(reps=1, skip=()):
    import concourse.bacc as bacc
    import concourse.tile as tile
    from concourse import mybir
    import concourse.bass as bass

    skip = set(skip)
    f32 = mybir.dt.float32
    f32r = mybir.dt.float32r
    bf16 = mybir.dt.bfloat16
    AF = mybir.ActivationFunctionType
    ALU = mybir.AluOpType

    nc = bacc.Bacc("TRN2", target_bir_lowering=False, debug=False,
                   num_devices=NCORES)

    # phi tiles and W replicated at 4 partition row-quads (32g..32g+9);
    # consecutive matmuls target disjoint PE quads via tile_position so
    # weight loads overlap the previous matmul's streaming.
    if MM4:
        phi_d = nc.dram_tensor("phi", [128, NT // 4, 128], bf16,
                               kind="ExternalInput")
        w_d = nc.dram_tensor("wmat", [128, NPOLY * 2 * K], bf16,
                             kind="ExternalInput")
    else:
        phi_d = nc.dram_tensor("phi", [9, NT, 128], bf16,
                               kind="ExternalInput")
        w_d = nc.dram_tensor("wmat", [9, NPOLY * 2 * K], bf16,
                             kind="ExternalInput")
    h_d = nc.dram_tensor("hrep", [128, BATCH, NPOLY, K], bf16,
                         kind="ExternalInput")
    sc_d = nc.dram_tensor("sc", [128, NT * NPOLY], f32, kind="ExternalOutput")
    mn_d = nc.dram_tensor("mn", [128, NT * NPOLY], f32, kind="ExternalOutput")

    import os
    KBUFS = int(os.environ.get("ANT_KBUFS", "3"))

    with tile.TileContext(nc) as tc:
        with (
            tc.tile_pool(name="const", bufs=1) as cpool,
            tc.tile_pool(name="psum", bufs=2, space="PSUM") as psum_pool,
            tc.tile_pool(name="work", bufs=KBUFS) as work,
            tc.tile_pool(name="outs", bufs=1) as outs,
        ):
            if MM4:
                phi_s = cpool.tile([128, NT // 4, 128], bf16)
                w_s = cpool.tile([128, NPOLY * 2 * K], bf16)
            else:
                phi_s = cpool.tile([9, NT, 128], bf16)
                w_s = cpool.tile([9, NPOLY * 2 * K], bf16)
            h_s = cpool.tile([128, BATCH, NPOLY, K], bf16)
            nc.sync.dma_start(phi_s[:], phi_d[:])
            nc.sync.dma_start(w_s[:], w_d[:])
            nc.sync.dma_start(h_s[:], h_d[:])

            sc_t = outs.tile([128, NT * NPOLY], f32)
            mn_t = outs.tile([128, NT * NPOLY], f32)

            def body():
                for b in range(NBATCH):
                    pt = psum_pool.tile([128, BATCH, NPOLY, 2, K], f32,
                                        tag="pt")
                    for t in MM_ORDER:
                        T = b * BATCH + t
                        if MM4:
                            g = T % 4
                            nc.tensor.matmul(
                                pt[:, t, :, :, :],
                                phi_s[32 * g:32 * g + 9, T // 4, :],
                                w_s[32 * g:32 * g + 9, :],
                                start=True, stop=True,
                                tile_position=(32 * g, 0),
                            )
                        else:
                            nc.tensor.matmul(
                                pt[:, t, :, :, :],
                                phi_s[:, T, :],
                                w_s[:],
                                start=True, stop=True,
                            )
                    n2 = pt[:, :, :, 0, :]
                    cross = pt[:, :, :, 1, :]
                    osl = slice(b * BATCH * NPOLY, (b + 1) * BATCH * NPOLY)

                    if "min" not in skip:
                        nc.vector.tensor_reduce(
                            mn_t[:, osl], n2,
                            axis=mybir.AxisListType.X, op=ALU.min)
                    if "tanh" not in skip:
                        t1 = work.tile([128, BATCH, NPOLY, K], bf16, tag="t1")
                        nc.scalar.activation(t1[:], cross, AF.Tanh,
                                             scale=100000.0)
                    if "mult" not in skip:
                        c_w = work.tile([128, BATCH, NPOLY, K], bf16, tag="c")
                        nc.vector.tensor_tensor(c_w[:], t1[:], h_s[:],
                                                op=ALU.mult)
                    if "add" not in skip:
                        nc.vector.tensor_reduce(
                            sc_t[:, osl], c_w[:],
                            axis=mybir.AxisListType.X, op=ALU.add)

            if reps > 1:
                with tc.For_i(0, reps, 1,
                              hint_engines=(mybir.EngineType.PE,
                                            mybir.EngineType.DVE)):
                    body()
            else:
                body()

            if "add" not in skip:
                nc.sync.dma_start(sc_d[:], sc_t[:])
            if "min" not in skip:
                nc.sync.dma_start(mn_d[:], mn_t[:])

    nc.compile()
    return nc


def _host_inputs(contour):
    """Per-core input maps: phi (pixel half), W + h tables (polygon pair)."""
    import ml_dtypes
    C = contour.reshape(NCORES, K, 2).astype(np.float64)

    ax = np.arange(SIZE) / SIZE
    m = np.arange(SIZE * SIZE)
    mx = (m // SIZE) / SIZE
    my = (m % SIZE) / SIZE
    # basis [1, mx, my]: all exactly representable in bf16 (8-bit grid).
    # The mx^2+my^2 term of n2 is constant across k, so min_k n2 =
    # min_k u_k + s with s added exactly on the host.
    phi_full = np.stack([np.ones_like(mx), mx, my], 0)

    maps = []
    for i in range(NCORES):
        a = i // 2
        half = i % 2
        polys = [2 * a, 2 * a + 1]

        wmat = np.zeros((3, NPOLY * 2 * K))
        hrep = np.zeros((128, BATCH, NPOLY, K))
        for pi, p in enumerate(polys):
            vx, vy = C[p, :, 0], C[p, :, 1]
            vxn, vyn = np.roll(vx, -1), np.roll(vy, -1)
            base = pi * 2 * K
            wmat[:, base:base + K] = np.stack(
                [vx * vx + vy * vy, -2 * vx, -2 * vy], 0)
            wmat[:, base + K:base + 2 * K] = np.stack(
                [vy * vxn - vx * vyn, vyn - vy, vx - vxn], 0)
            # h table over all 256 my values
            dy = vy[None, :] - ax[:, None]      # (256, K)
            dyn = vyn[None, :] - ax[:, None]
            H = (np.sign(dy) != np.sign(dyn)).astype(np.float64)
            for t in range(BATCH):
                hrep[:, t, pi, :] = H[(t % 2) * 128:(t % 2) * 128 + 128, :]

        wmat = wmat.astype(np.float32).astype(np.float64)
        sl = slice(half * 32768, (half + 1) * 32768)
        phi = phi_full[:, sl].astype(np.float32).astype(np.float64)

        # phi rows are exact in bf16; W gets a 3-way bf16 split so one K=9
        # bf16 matmul reproduces the fp32 matmul to ~2^-24.
        def split3(x):
            h = x.astype(ml_dtypes.bfloat16).astype(np.float64)
            r = x - h
            mi = r.astype(ml_dtypes.bfloat16).astype(np.float64)
            lo = (r - mi).astype(ml_dtypes.bfloat16).astype(np.float64)
            return h, mi, lo

        wh, wm, wl = split3(wmat)
        phi9 = np.concatenate([phi, phi, phi], 0).reshape(9, NT, 128)
        w9 = np.concatenate([wh, wm, wl], 0)
        if MM4:
            phi_a = np.zeros((128, NT // 4, 128))
            w_a = np.zeros((128, NPOLY * 2 * K))
            for g in range(4):
                w_a[32 * g:32 * g + 9, :] = w9
            for T in range(NT):
                g = T % 4
                phi_a[32 * g:32 * g + 9, T // 4, :] = phi9[:, T, :]
        else:
            phi_a, w_a = phi9, w9
        maps.append({
            "phi": phi_a.astype(ml_dtypes.bfloat16),
            "wmat": w_a.astype(ml_dtypes.bfloat16),
            "hrep": hrep.astype(ml_dtypes.bfloat16),
        })
    return maps


def _get_executor(reps=1, skip=()):
    """Build (once) a reusable jitted SPMD executor over the 8 cores."""
    key = ("exec", reps, tuple(sorted(skip)))
    if key in _CACHE:
        return _CACHE[key]

    import jax
    from jax.sharding import Mesh, PartitionSpec, NamedSharding
    from jax.experimental.shard_map import shard_map
    import concourse.mybir as mybir
    from concourse.bass2jax import _bass_exec_p, install_neuronx_cc_hook

    install_neuronx_cc_hook()
    nckey = ("nc", reps, tuple(sorted(skip)))
    if nckey not in _CACHE:
        _CACHE[nckey] = _build_program(reps=reps, skip=skip)
    nc = _CACHE[nckey]
    partition_name = (nc.partition_id_tensor.name
                      if nc.partition_id_tensor else None)

    in_names, out_names, out_avals, zero_outs = [], [], [], []
    for alloc in nc.m.functions[0].allocations:
        if not isinstance(alloc, mybir.MemoryLocationSet):
            continue
        name = alloc.memorylocations[0].name
        if alloc.kind == "ExternalInput":
            if name == partition_name:
                continue
            in_names.append(name)
        elif alloc.kind == "ExternalOutput":
            out_names.append(name)
            shape = tuple(alloc.tensor_shape)
            dtype = mybir.dt.np(alloc.dtype)
            out_avals.append(jax.core.ShapedArray(shape, dtype))
            zero_outs.append(np.zeros(shape, dtype))
    n_params = len(in_names)
    all_names = in_names + out_names
    if partition_name is not None:
        all_names = all_names + [partition_name]

    from concourse.bass2jax import partition_id_tensor

    def _body(*args):
        operands = list(args)
        if partition_name is not None:
            operands.append(partition_id_tensor())
        outs = _bass_exec_p.bind(
            *operands,
            out_avals=tuple(out_avals),
            in_names=tuple(all_names),
            out_names=tuple(out_names),
            lowering_input_output_aliases=(),
            sim_require_finite=True,
            sim_require_nnan=True,
            nc=nc,
        )
        return tuple(outs)

    devices = jax.devices()[:NCORES]
    mesh = Mesh(np.asarray(devices), ("core",))
    nspec = (PartitionSpec("core"),) * (n_params + len(out_names))
    sharded = jax.jit(
        shard_map(_body, mesh=mesh, in_specs=nspec,
                  out_specs=(PartitionSpec("core"),) * len(out_names),
                  check_rep=False),
        keep_unused=True,
    )
    sharding = NamedSharding(mesh, PartitionSpec("core"))
    zeros_dev = [
        jax.device_put(
            np.zeros((NCORES * z.shape[0], *z.shape[1:]), z.dtype), sharding)
        for z in zero_outs
    ]
    _CACHE[key] = (sharded, sharding, in_names, out_names, zeros_dev)
    return _CACHE[key]


def _run(contour):
    """Returns list (per core) of dicts {sc, mn} as np arrays."""
    import jax
    sharded, sharding, in_names, out_names, zeros_dev = _get_executor()
    in_maps = _host_inputs(contour)
    concat = {
        name: np.concatenate([m[name] for m in in_maps], axis=0)
        for name in in_names
    }
    if "phi_dev" not in _CACHE:
        _CACHE["phi_dev"] = jax.device_put(concat["phi"], sharding)
    ins = [
        _CACHE["phi_dev"] if name == "phi"
        else jax.device_put(concat[name], sharding)
        for name in in_names
    ]
    outs = sharded(*ins, *zeros_dev)
    res = []
    for c in range(NCORES):
        d = {}
        for i, name in enumerate(out_names):
            arr = np.asarray(outs[i])
            rows = arr.shape[0] // NCORES
            d[name] = arr[c * rows:(c + 1) * rows]
        res.append(d)
    return res


def benchmark(contour, iters=20, reps=1, skip=()):
    """Pipelined repeated execution; returns avg seconds/iteration."""
    import time
    import jax
    sharded, sharding, in_names, out_names, zeros_dev = _get_executor(
        reps, skip)
    in_maps = _host_inputs(np.asarray(contour, dtype=np.float32))
    concat = {
        name: np.concatenate([m[name] for m in in_maps], axis=0)
        for name in in_names
    }
    ins = [jax.device_put(concat[name], sharding) for name in in_names]
    out = sharded(*ins, *zeros_dev)  # warm-up
    jax.block_until_ready(out)
    t0 = time.time()
    outs = [sharded(*ins, *zeros_dev) for _ in range(iters)]
    jax.block_until_ready(outs[-1])
    t1 = time.time()
    return (t1 - t0) / iters


def kernel(contour, *, _trace=False):
    contour = np.asarray(contour, dtype=np.float32)
    results = _run(contour)

    m = np.arange(SIZE * SIZE)
    s_full = (((m // SIZE) / SIZE) ** 2 + ((m % SIZE) / SIZE) ** 2).astype(
        np.float64)

    prod = np.zeros((NCORES, SIZE * SIZE), np.float32)
    for i in range(NCORES):
        a = i // 2
        half = i % 2
        sl = slice(half * 32768, (half + 1) * 32768)
        S = results[i]["sc"].reshape(128, NT, NPOLY)
        M = results[i]["mn"].reshape(128, NT, NPOLY)
        for pi in range(NPOLY):
            wind = np.abs(S[:, :, pi].T.ravel()) * np.float32(0.5)
            n2 = M[:, :, pi].T.ravel() + s_full[sl]
            dist = np.sqrt(np.maximum(n2, 0.0))
            prod[2 * a + pi, sl] = wind * dist
    dmap = (prod / prod.max()).astype(np.float32)
    return dmap.reshape(2, 4, SIZE, SIZE)


# revision 29
# speedup vs baseline: 1.0245x; 1.0245x over previous
"""Contour -> distance map kernel for 8 Trainium2 NeuronCores.

Math (per polygon, pixel m=(mx,my), edge k: vertex a=v_k, next b=v_{k+1}):
  cross_k = (a-m) x (b-m)   (affine in [1, mx, my])
  n2_k    = |a-m|^2         (affine in phi(m) = [1, mx, my, mx^2+my^2])
The reference's tanh/arccos winding-angle sum equals (a.e.) the integer
winding number, which the kernel computes by signed ray-crossing counting:
  W = 1/2 * sum_k tanh(1e5*cross_k) * h_k,
  h_k = [sgn(vy_k - my) != sgn(vy_{k+1} - my)]  (edge spans the pixel row).
h depends only on (edge, my); my has a fixed 128-lane pattern per pixel-tile
parity, so h is a host-precomputed constant SBUF table - no dot products,
reciprocals or arctans on device (CPU-checked rel err 2.7e-4 vs reference).

Sharding: core i handles polygon pair {2*(i//2), 2*(i//2)+1} on pixel half
i%2 (32768 px).  Per 128-pixel tile ONE fp32r matmul (1 cycle/col at >=256
out cols) yields psum[128, (poly, kind, 64)] = n2|cross for both polygons.
Then: ACT tanh(1e5*cross)->bf16, DVE c = t1*h (bf16 2x mode), DVE add-reduce
c -> winding sum, DVE min-reduce n2.  Host epilogue: wind = |SC|/2,
prod = wind*sqrt(MN), dmap = prod / global_max(prod).
"""

import numpy as np

SIZE = 256
NCORES = 8
K = 64

import os as _os
# NPOLY polygons per core, each core covering 65536/NPOLY pixels: larger
# NPOLY = fewer, fatter matmuls (the HW has a ~600ns fixed cost per matmul).
NPOLY = int(_os.environ.get("ANT_NPOLY", "2"))
NT = 512 // NPOLY    # 128-pixel tiles per core
BATCH = 16 // NPOLY  # tiles per psum batch (psum: BATCH*NPOLY*2*64 cols)
NBATCH = NT // BATCH # = 32
PIX = NT * 128       # pixels per core
MM4 = _os.environ.get("ANT_MM4", "0") == "1"
MM_ORDER = [int(x) for x in _os.environ["ANT_MM_ORDER"].split(",")] \
    if "ANT_MM_ORDER" in _os.environ else list(range(BATCH))

_CACHE = {}


def _build_program(reps=1, skip=()):
    import concourse.bacc as bacc
    import concourse.tile as tile
    from concourse import mybir
    import concourse.bass as bass

    skip = set(skip)
    f32 = mybir.dt.float32
    f32r = mybir.dt.float32r
    bf16 = mybir.dt.bfloat16
    AF = mybir.ActivationFunctionType
    ALU = mybir.AluOpType

    nc = bacc.Bacc("TRN2", target_bir_lowering=False, debug=False,
                   num_devices=NCORES)

    # phi tiles and W replicated at 4 partition row-quads (32g..32g+9);
    # consecutive matmuls target disjoint PE quads via tile_position so
    # weight loads overlap the previous matmul's streaming.
    if MM4:
        phi_d = nc.dram_tensor("phi", [128, NT // 4, 128], bf16,
                               kind="ExternalInput")
        w_d = nc.dram_tensor("wmat", [128, NPOLY * 2 * K], bf16,
                             kind="ExternalInput")
    else:
        phi_d = nc.dram_tensor("phi", [9, NT, 128], bf16,
                               kind="ExternalInput")
        w_d = nc.dram_tensor("wmat", [9, NPOLY * 2 * K], bf16,
                             kind="ExternalInput")
    h_d = nc.dram_tensor("hrep", [128, BATCH, NPOLY, K], bf16,
                         kind="ExternalInput")
    sc_d = nc.dram_tensor("sc", [128, NT * NPOLY], f32, kind="ExternalOutput")
    mn_d = nc.dram_tensor("mn", [128, NT * NPOLY], f32, kind="ExternalOutput")

    import os
    KBUFS = int(os.environ.get("ANT_KBUFS", "3"))

    with tile.TileContext(nc) as tc:
        with (
            tc.tile_pool(name="const", bufs=1) as cpool,
            tc.tile_pool(name="psum", bufs=2, space="PSUM") as psum_pool,
            tc.tile_pool(name="work", bufs=KBUFS) as work,
            tc.tile_pool(name="outs", bufs=1) as outs,
        ):
            if MM4:
                phi_s = cpool.tile([128, NT // 4, 128], bf16)
                w_s = cpool.tile([128, NPOLY * 2 * K], bf16)
            else:
                phi_s = cpool.tile([9, NT, 128], bf16)
                w_s = cpool.tile([9, NPOLY * 2 * K], bf16)
            h_s = cpool.tile([128, BATCH, NPOLY, K], bf16)
            nc.sync.dma_start(phi_s[:], phi_d[:])
            nc.sync.dma_start(w_s[:], w_d[:])
            nc.sync.dma_start(h_s[:], h_d[:])

            sc_t = outs.tile([128, NT * NPOLY], f32)
            mn_t = outs.tile([128, NT * NPOLY], f32)

            def body():
                for b in range(NBATCH):
                    pt = psum_pool.tile([128, BATCH, NPOLY, 2, K], f32,
                                        tag="pt")
                    for t in MM_ORDER:
                        T = b * BATCH + t
                        if MM4:
                            g = T % 4
                            nc.tensor.matmul(
                                pt[:, t, :, :, :],
                                phi_s[32 * g:32 * g + 9, T // 4, :],
                                w_s[32 * g:32 * g + 9, :],
                                start=True, stop=True,
                                tile_position=(32 * g, 0),
                            )
                        else:
                            nc.tensor.matmul(
                                pt[:, t, :, :, :],
                                phi_s[:, T, :],
                                w_s[:],
                                start=True, stop=True,
                            )
                    n2 = pt[:, :, :, 0, :]
                    cross = pt[:, :, :, 1, :]
                    osl = slice(b * BATCH * NPOLY, (b + 1) * BATCH * NPOLY)

                    if "min" not in skip:
                        nc.vector.tensor_reduce(
                            mn_t[:, osl], n2,
                            axis=mybir.AxisListType.X, op=ALU.min)
                    if "tanh" not in skip:
                        t1 = work.tile([128, BATCH, NPOLY, K], bf16, tag="t1")
                        nc.scalar.activation(t1[:], cross, AF.Tanh,
                                             scale=100000.0)
                    if "mult" not in skip:
                        c_w = work.tile([128, BATCH, NPOLY, K], bf16, tag="c")
                        nc.vector.tensor_tensor(c_w[:], t1[:], h_s[:],
                                                op=ALU.mult)
                    if "add" not in skip:
                        nc.vector.tensor_reduce(
                            sc_t[:, osl], c_w[:],
                            axis=mybir.AxisListType.X, op=ALU.add)

            if reps > 1:
                with tc.For_i(0, reps, 1,
                              hint_engines=(mybir.EngineType.PE,
                                            mybir.EngineType.DVE)):
                    body()
            else:
                body()

            if "add" not in skip:
                nc.sync.dma_start(sc_d[:], sc_t[:])
            if "min" not in skip:
                nc.sync.dma_start(mn_d[:], mn_t[:])

    nc.compile()
    return nc


def _host_inputs(contour):
    """Per-core input maps: phi (pixel half), W + h tables (polygon pair)."""
    import ml_dtypes
    C = contour.reshape(NCORES, K, 2).astype(np.float64)

    ax = np.arange(SIZE) / SIZE
    m = np.arange(SIZE * SIZE)
    mx = (m // SIZE) / SIZE
    my = (m % SIZE) / SIZE
    # basis [1, mx, my]: all exactly representable in bf16 (8-bit grid).
    # The mx^2+my^2 term of n2 is constant across k, so min_k n2 =
    # min_k u_k + s with s added exactly on the host.
    phi_full = np.stack([np.ones_like(mx), mx, my], 0)

    maps = []
    for i in range(NCORES):
        pset = i // NPOLY       # polygon-set index
        blk = i % NPOLY         # pixel-block index
        polys = list(range(pset * NPOLY, (pset + 1) * NPOLY))

        wmat = np.zeros((3, NPOLY * 2 * K))
        hrep = np.zeros((128, BATCH, NPOLY, K))
        for pi, p in enumerate(polys):
            vx, vy = C[p, :, 0], C[p, :, 1]
            vxn, vyn = np.roll(vx, -1), np.roll(vy, -1)
            base = pi * 2 * K
            wmat[:, base:base + K] = np.stack(
                [vx * vx + vy * vy, -2 * vx, -2 * vy], 0)
            wmat[:, base + K:base + 2 * K] = np.stack(
                [vy * vxn - vx * vyn, vyn - vy, vx - vxn], 0)
            # h table over all 256 my values
            dy = vy[None, :] - ax[:, None]      # (256, K)
            dyn = vyn[None, :] - ax[:, None]
            H = (np.sign(dy) != np.sign(dyn)).astype(np.float64)
            for t in range(BATCH):
                hrep[:, t, pi, :] = H[(t % 2) * 128:(t % 2) * 128 + 128, :]

        wmat = wmat.astype(np.float32).astype(np.float64)
        sl = slice(blk * PIX, (blk + 1) * PIX)
        phi = phi_full[:, sl].astype(np.float32).astype(np.float64)

        # phi rows are exact in bf16; W gets a 3-way bf16 split so one K=9
        # bf16 matmul reproduces the fp32 matmul to ~2^-24.
        def split3(x):
            h = x.astype(ml_dtypes.bfloat16).astype(np.float64)
            r = x - h
            mi = r.astype(ml_dtypes.bfloat16).astype(np.float64)
            lo = (r - mi).astype(ml_dtypes.bfloat16).astype(np.float64)
            return h, mi, lo

        wh, wm, wl = split3(wmat)
        phi9 = np.concatenate([phi, phi, phi], 0).reshape(9, NT, 128)
        w9 = np.concatenate([wh, wm, wl], 0)
        if MM4:
            phi_a = np.zeros((128, NT // 4, 128))
            w_a = np.zeros((128, NPOLY * 2 * K))
            for g in range(4):
                w_a[32 * g:32 * g + 9, :] = w9
            for T in range(NT):
                g = T % 4
                phi_a[32 * g:32 * g + 9, T // 4, :] = phi9[:, T, :]
        else:
            phi_a, w_a = phi9, w9
        maps.append({
            "phi": phi_a.astype(ml_dtypes.bfloat16),
            "wmat": w_a.astype(ml_dtypes.bfloat16),
            "hrep": hrep.astype(ml_dtypes.bfloat16),
        })
    return maps


def _get_executor(reps=1, skip=()):
    """Build (once) a reusable jitted SPMD executor over the 8 cores."""
    key = ("exec", reps, tuple(sorted(skip)))
    if key in _CACHE:
        return _CACHE[key]

    import jax
    from jax.sharding import Mesh, PartitionSpec, NamedSharding
    from jax.experimental.shard_map import shard_map
    import concourse.mybir as mybir
    from concourse.bass2jax import _bass_exec_p, install_neuronx_cc_hook

    install_neuronx_cc_hook()
    nckey = ("nc", reps, tuple(sorted(skip)))
    if nckey not in _CACHE:
        _CACHE[nckey] = _build_program(reps=reps, skip=skip)
    nc = _CACHE[nckey]
    partition_name = (nc.partition_id_tensor.name
                      if nc.partition_id_tensor else None)

    in_names, out_names, out_avals, zero_outs = [], [], [], []
    for alloc in nc.m.functions[0].allocations:
        if not isinstance(alloc, mybir.MemoryLocationSet):
            continue
        name = alloc.memorylocations[0].name
        if alloc.kind == "ExternalInput":
            if name == partition_name:
                continue
            in_names.append(name)
        elif alloc.kind == "ExternalOutput":
            out_names.append(name)
            shape = tuple(alloc.tensor_shape)
            dtype = mybir.dt.np(alloc.dtype)
            out_avals.append(jax.core.ShapedArray(shape, dtype))
            zero_outs.append(np.zeros(shape, dtype))
    n_params = len(in_names)
    all_names = in_names + out_names
    if partition_name is not None:
        all_names = all_names + [partition_name]

    from concourse.bass2jax import partition_id_tensor

    def _body(*args):
        operands = list(args)
        if partition_name is not None:
            operands.append(partition_id_tensor())
        outs = _bass_exec_p.bind(
            *operands,
            out_avals=tuple(out_avals),
            in_names=tuple(all_names),
            out_names=tuple(out_names),
            lowering_input_output_aliases=(),
            sim_require_finite=True,
            sim_require_nnan=True,
            nc=nc,
        )
        return tuple(outs)

    devices = jax.devices()[:NCORES]
    mesh = Mesh(np.asarray(devices), ("core",))
    nspec = (PartitionSpec("core"),) * (n_params + len(out_names))
    sharded = jax.jit(
        shard_map(_body, mesh=mesh, in_specs=nspec,
                  out_specs=(PartitionSpec("core"),) * len(out_names),
                  check_rep=False),
        keep_unused=True,
    )
    sharding = NamedSharding(mesh, PartitionSpec("core"))
    zeros_dev = [
        jax.device_put(
            np.zeros((NCORES * z.shape[0], *z.shape[1:]), z.dtype), sharding)
        for z in zero_outs
    ]
    _CACHE[key] = (sharded, sharding, in_names, out_names, zeros_dev)
    return _CACHE[key]


def _run(contour):
    """Returns list (per core) of dicts {sc, mn} as np arrays."""
    import jax
    sharded, sharding, in_names, out_names, zeros_dev = _get_executor()
    in_maps = _host_inputs(contour)
    concat = {
        name: np.concatenate([m[name] for m in in_maps], axis=0)
        for name in in_names
    }
    if "phi_dev" not in _CACHE:
        _CACHE["phi_dev"] = jax.device_put(concat["phi"], sharding)
    ins = [
        _CACHE["phi_dev"] if name == "phi"
        else jax.device_put(concat[name], sharding)
        for name in in_names
    ]
    outs = sharded(*ins, *zeros_dev)
    res = []
    for c in range(NCORES):
        d = {}
        for i, name in enumerate(out_names):
            arr = np.asarray(outs[i])
            rows = arr.shape[0] // NCORES
            d[name] = arr[c * rows:(c + 1) * rows]
        res.append(d)
    return res


def benchmark(contour, iters=20, reps=1, skip=()):
    """Pipelined repeated execution; returns avg seconds/iteration."""
    import time
    import jax
    sharded, sharding, in_names, out_names, zeros_dev = _get_executor(
        reps, skip)
    in_maps = _host_inputs(np.asarray(contour, dtype=np.float32))
    concat = {
        name: np.concatenate([m[name] for m in in_maps], axis=0)
        for name in in_names
    }
    ins = [jax.device_put(concat[name], sharding) for name in in_names]
    out = sharded(*ins, *zeros_dev)  # warm-up
    jax.block_until_ready(out)
    t0 = time.time()
    outs = [sharded(*ins, *zeros_dev) for _ in range(iters)]
    jax.block_until_ready(outs[-1])
    t1 = time.time()
    return (t1 - t0) / iters


def kernel(contour, *, _trace=False):
    contour = np.asarray(contour, dtype=np.float32)
    results = _run(contour)

    m = np.arange(SIZE * SIZE)
    s_full = (((m // SIZE) / SIZE) ** 2 + ((m % SIZE) / SIZE) ** 2).astype(
        np.float64)

    prod = np.zeros((NCORES, SIZE * SIZE), np.float32)
    for i in range(NCORES):
        pset = i // NPOLY
        blk = i % NPOLY
        sl = slice(blk * PIX, (blk + 1) * PIX)
        S = results[i]["sc"].reshape(128, NT, NPOLY)
        M = results[i]["mn"].reshape(128, NT, NPOLY)
        for pi in range(NPOLY):
            wind = np.abs(S[:, :, pi].T.ravel()) * np.float32(0.5)
            n2 = M[:, :, pi].T.ravel() + s_full[sl]
            dist = np.sqrt(np.maximum(n2, 0.0))
            prod[pset * NPOLY + pi, sl] = wind * dist
    dmap = (prod / prod.max()).astype(np.float32)
    return dmap.reshape(2, 4, SIZE, SIZE)
